# revision 9
# baseline (speedup 1.0000x reference)
"""Multi-head causal attention (B=4, S=2048, H=1024, NH=16) on 8 trn2 cores.

Hybrid sharding: core = (batch b, head-half hh) -> 1 batch x 8 heads per
core.  fp8 DoubleRow matmuls carry the projections (hi/lo e4m3+e5m2
compensation: Q/K 2 passes, V 3) and the Q*K score matmuls (e4m3 stores);
P@V / output projection run bf16.  Scores are computed transposed
S^T[k,q] so softmax denominators fall out of a ones-column in the P@V
accumulation and normalization is a per-partition scalar multiply.  The
ctx^T -> ctx transpose uses the DMA xbar (no PE/PSUM involved), and the
output projection contracts all 512 local channels so each core emits a
[2048,1024] partial for its batch; the host sums the 2 partials per
batch and folds in bo + bv@Wo^T.  Slot order is j-outer/h-inner so
transposes + output projection pipeline level-by-level."""
import numpy as np
import ml_dtypes

import concourse.bacc as bacc
import concourse.tile as tile
from concourse import mybir
from concourse.bass_utils import run_bass_kernel_spmd

F32 = mybir.dt.float32
BF16 = mybir.dt.bfloat16
E4 = mybir.dt.float8e4
E5 = mybir.dt.float8e5
AF = mybir.ActivationFunctionType
DR = mybir.MatmulPerfMode.DoubleRow
MULT = mybir.AluOpType.mult
ADD = mybir.AluOpType.add

B, S, H, NH = 4, 2048, 1024, 16
HD = H // NH            # 64
NCORES = 8
HPC = 8                 # heads per core
C = HPC * HD            # 512 channels per core
SCALE = 1.0 / np.sqrt(HD)
WS = 32.0               # weight pre-scale (keeps fp8 hi in normal range)

N_J = S // 256          # 8 j-levels of 256 q each
QK_PASSES = 2
V_PASSES = 3

_CACHE = {}


def _build_nc():
    nc = bacc.Bacc(name="mha_v2")
    xh_d = nc.dram_tensor("xh", [H, S], E4, kind="ExternalInput")
    xl_d = nc.dram_tensor("xl", [H, S], E5, kind="ExternalInput")
    wqh_d = nc.dram_tensor("wqh", [128, 4096], E4, kind="ExternalInput")
    wql_d = nc.dram_tensor("wql", [128, 4096], E5, kind="ExternalInput")
    wkh_d = nc.dram_tensor("wkh", [128, 4096], E4, kind="ExternalInput")
    wkl_d = nc.dram_tensor("wkl", [128, 4096], E5, kind="ExternalInput")
    wvh_d = nc.dram_tensor("wvh", [128, 4096], E4, kind="ExternalInput")
    wvl_d = nc.dram_tensor("wvl", [128, 4096], E5, kind="ExternalInput")
    wo_d = nc.dram_tensor("wo", [128, 4096], BF16, kind="ExternalInput")
    bq_d = nc.dram_tensor("bq", [128, 4], F32, kind="ExternalInput")
    bk_d = nc.dram_tensor("bk", [128, 4], F32, kind="ExternalInput")
    tri_d = nc.dram_tensor("tri", [128, 128], BF16, kind="ExternalInput")
    out_d = nc.dram_tensor("out", [S, H], BF16, kind="ExternalOutput")

    with tile.TileContext(nc) as tc:
        with (
            tc.tile_pool(name="const", bufs=1) as cp,
            tc.tile_pool(name="xs", bufs=1) as xp,
            tc.tile_pool(name="qk", bufs=1) as qp,
            tc.tile_pool(name="vn", bufs=1) as vp,
            tc.tile_pool(name="pts", bufs=10) as pp,
            tc.tile_pool(name="ctx", bufs=1) as ctp,
            tc.tile_pool(name="small", bufs=8) as sp,
            tc.tile_pool(name="osb", bufs=3) as op_,
            tc.tile_pool(name="stp", bufs=2, space="PSUM") as stp,
            tc.tile_pool(name="accp", bufs=1, space="PSUM") as accp,
            tc.tile_pool(name="mixp", bufs=3, space="PSUM") as mixp,
        ):
            # ---- constants ----
            wqh_s = cp.tile([128, 4096], E4)
            wql_s = cp.tile([128, 4096], E5)
            wkh_s = cp.tile([128, 4096], E4)
            wkl_s = cp.tile([128, 4096], E5)
            wvh_s = cp.tile([128, 4096], E4)
            wvl_s = cp.tile([128, 4096], E5)
            wo_s = cp.tile([128, 4096], BF16)
            bq_s = cp.tile([128, 4], F32)
            bk_s = cp.tile([128, 4], F32)
            tri_s = cp.tile([128, 128], BF16)
            for s_, d_ in ((wqh_s, wqh_d), (wql_s, wql_d), (bq_s, bq_d),
                           (wkh_s, wkh_d), (wkl_s, wkl_d), (bk_s, bk_d),
                           (tri_s, tri_d)):
                nc.sync.dma_start(s_[:], d_.ap())

            # x tiles (one batch): [128 p, 8 c, 2048 t], ch = c*128 + p
            xh_s = xp.tile([128, 16384], E4)
            xl_s = xp.tile([128, 16384], E5)
            for half in range(2):
                hsl = slice(half * 1024, (half + 1) * 1024)
                for s_, d_ in ((xh_s, xh_d), (xl_s, xl_d)):
                    nc.sync.dma_start(
                        s_.rearrange("p (c t) -> p c t", c=8)[:, :, hsl],
                        d_.ap()[:, hsl].rearrange("(c p) t -> p c t", p=128))

            for s_, d_ in ((wvh_s, wvh_d), (wvl_s, wvl_d), (wo_s, wo_d)):
                nc.sync.dma_start(s_[:], d_.ap())

            # weight views [p, c2, i, out512]
            wv_ = {
                "qh": wqh_s.rearrange("p (c i o) -> p c i o", c=4, i=2),
                "ql": wql_s.rearrange("p (c i o) -> p c i o", c=4, i=2),
                "kh": wkh_s.rearrange("p (c i o) -> p c i o", c=4, i=2),
                "kl": wkl_s.rearrange("p (c i o) -> p c i o", c=4, i=2),
                "vh": wvh_s.rearrange("p (c i o) -> p c i o", c=4, i=2),
                "vl": wvl_s.rearrange("p (c i o) -> p c i o", c=4, i=2),
            }
            wo_v = wo_s.rearrange("p (cc o) -> p cc o", cc=4)
            xh_v = xh_s.rearrange("p (c i t) -> p c i t", c=4, i=2)
            xl_v = xl_s.rearrange("p (c i t) -> p c i t", c=4, i=2)

            # q/k channel-major tiles per cc chunk (2 heads each)
            qt = [qp.tile([128, 2048], E4, name=f"qt{cc}") for cc in range(4)]
            kt = [qp.tile([128, 2048], E4, name=f"kt{cc}") for cc in range(4)]
            # DR-packed q/k: [64 (2h x 32), 2 i, 2048 t] per head-pair cc
            qdr = [qp.tile([64, 4096], E4, name=f"qdr{g}") for g in range(4)]
            kdr = [qp.tile([64, 4096], E4, name=f"kdr{g}") for g in range(4)]
            # v (+ones col): [128 k-part, 8 h, 16 kc, 65]
            vn = vp.tile([128, 8320], BF16)
            vn4 = vn.rearrange("p (h c e) -> p h c e", h=8, e=65)
            nc.vector.memset(vn4[:, :, :, 64], 1.0)
            # ctx^T [128 tok, 16 qc, 8 h, 64 d]; ctx [128 chp, 4 cc, 2048 t]
            ctxT = ctp.tile([128, 8192], BF16)
            ctx = ctp.tile([128, 8192], BF16)
            ctx3 = ctx.rearrange("p (cc t) -> p cc t", cc=4)
            acc = accp.tile([128, 512], F32)
            accv = acc.rearrange("p (s q e) -> p s q e", s=2, q=2)

            def emit_qkproj(cc, st):
                tsl = slice(st * 512, (st + 1) * 512)
                for wh, wl, bias, dst in (("qh", "ql", bq_s, qt),
                                          ("kh", "kl", bk_s, kt)):
                    pmm = mixp.tile([128, 512], F32, tag="mix",
                                    name=f"pp{wh}{cc}_{st}")
                    first = True
                    for w_v in (wv_[wh], wv_[wl])[:QK_PASSES]:
                        for c2 in range(4):
                            nc.tensor.matmul(
                                pmm[:],
                                w_v[:, c2, :, cc * 128:(cc + 1) * 128],
                                xh_v[:, c2, :, tsl],
                                start=first,
                                stop=(w_v is wv_[wl] and c2 == 3),
                                perf_mode=DR)
                            first = False
                    nc.vector.tensor_scalar(dst[cc][:, tsl], pmm[:], 1.0 / WS,
                                            bias[:, cc:cc + 1], op0=MULT,
                                            op1=ADD)

            def emit_remap(cc, t0, t1):
                tsl = slice(t0, t1)
                for src_l, dr in ((qt, qdr), (kt, kdr)):
                    drv = dr[cc].rearrange("p (i t) -> p i t", i=2)
                    for hl in range(2):
                        p0 = hl * 32
                        for i in range(2):
                            nc.sync.dma_start(
                                drv[p0:p0 + 32, i, tsl],
                                src_l[cc][hl * 64 + i * 32:
                                          hl * 64 + i * 32 + 32, tsl])

            def emit_vproj(c):
                # flipped V: psum [128 tok, 512 ch] for one 128-token chunk
                tsl = slice(c * 128, (c + 1) * 128)
                vm = mixp.tile([128, 512], F32, tag="mix", name=f"vp{c}")
                passes = [("vh", xh_v), ("vl", xh_v)]
                if V_PASSES >= 3:
                    passes.append(("vh", xl_v))
                first = True
                for wn, x_v in passes:
                    for c2 in range(4):
                        nc.tensor.matmul(
                            vm[:], x_v[:, c2, :, tsl], wv_[wn][:, c2],
                            start=first,
                            stop=(wn == passes[-1][0] and x_v is passes[-1][1]
                                  and c2 == 3),
                            perf_mode=DR)
                        first = False
                nc.vector.tensor_scalar(
                    vn4[:, :, c, 0:64],
                    vm.rearrange("p (h d) -> p h d", h=8), 1.0 / WS, None,
                    op0=MULT)

            def group_chunks(j):
                nkc = 2 * (j + 1)
                groups = []
                for g0 in range(0, nkc, 4):
                    cs = list(range(g0, min(g0 + 4, nkc)))
                    pos, w = [], []
                    for c in cs:
                        pos.append((c - g0) * 256)
                        w.append(128 if c == nkc - 1 else 256)
                    groups.append((cs, w, pos, pos[-1] + w[-1]))
                return groups

            def emit_qk_scores(h, j, g):
                qv = qdr[h // 2].rearrange("p (i t) -> p i t", i=2)
                kv = kdr[h // 2].rearrange("p (i t) -> p i t", i=2)
                hsl = slice((h % 2) * 32, (h % 2) * 32 + 32)
                cs, ws, poss, tw = g
                st = stp.tile([128, 1024], F32, tag="st",
                              name=f"st{h}_{j}_{cs[0]}")
                nkc = 2 * (j + 1)
                for c, w, pos in zip(cs, ws, poss):
                    q0 = j * 256 + (128 if c == nkc - 1 else 0)
                    nc.tensor.matmul(
                        st[:, pos:pos + w],
                        kv[hsl, :, c * 128:(c + 1) * 128],
                        qv[hsl, :, q0:q0 + w],
                        start=True, stop=True, perf_mode=DR)
                return st

            def emit_exp(h, j, g, st):
                cs, ws, poss, tw = g
                pt = pp.tile([128, 1024], BF16, tag="pt",
                             name=f"pt{h}_{j}_{cs[0]}")
                nc.scalar.activation(pt[:, 0:tw], st[:, 0:tw], AF.Exp,
                                     scale=float(SCALE))
                return pt

            def emit_mask(h, j, pts_map):
                nkc = 2 * (j + 1)
                for c in (nkc - 2, nkc - 1):
                    pt = pts_map[c // 4]
                    pos = (c % 4) * 256
                    nc.gpsimd.tensor_mul(pt[:, pos:pos + 128],
                                         pt[:, pos:pos + 128], tri_s[:])

            def emit_pv(h, j, pts_map, sl):
                # sl: acc half (0 or 256) for this slot
                nkc = 2 * (j + 1)
                for qb in range(2):
                    qc = 2 * j + qb
                    for c in range(qc + 1):
                        pt = pts_map[c // 4]
                        pos = (c % 4) * 256 + (qb * 128 if c < nkc - 1 else 0)
                        nc.tensor.matmul(
                            acc[:, sl + qb * 128: sl + qb * 128 + 65],
                            pt[:, pos:pos + 128],
                            vn[:, h * 1040 + c * 65: h * 1040 + (c + 1) * 65],
                            start=(c == 0), stop=(c == qc),
                            skip_group_check=True)
                den = sp.tile([128, 2], F32, tag="den", name=f"dn{h}_{j}")
                s2 = sl // 256
                nc.vector.reciprocal(den[:], accv[:, s2, :, 64])
                for qb in range(2):
                    qc = 2 * j + qb
                    nc.vector.tensor_scalar(
                        ctxT[:, qc * 512 + h * 64: qc * 512 + (h + 1) * 64],
                        acc[:, sl + qb * 128: sl + qb * 128 + 64],
                        den[:, qb:qb + 1], None, op0=MULT)

            def emit_trans(qc):
                nc.sync.dma_start_transpose(
                    ctx3[:, :, qc * 128:(qc + 1) * 128],
                    ctxT[:, qc * 512:(qc + 1) * 512])

            def emit_oproj(qc):
                osb = op_.tile([128, 1024], BF16, tag="osb", name=f"ob{qc}")
                for half in range(2):
                    om = mixp.tile([128, 512], F32, tag="mix",
                                   name=f"om{qc}_{half}")
                    for cc in range(4):
                        nc.tensor.matmul(
                            om[:],
                            ctx3[:, cc, qc * 128:(qc + 1) * 128],
                            wo_v[:, cc, half * 512:(half + 1) * 512],
                            start=(cc == 0), stop=(cc == 3))
                    nc.vector.tensor_copy(
                        osb[:, half * 512:(half + 1) * 512], om[:])
                nc.sync.dma_start(
                    out_d.ap()[qc * 128:(qc + 1) * 128, :], osb[:])

            # ---------------- emission schedule ----------------
            for cc in range(4):
                emit_qkproj(cc, 0)
                emit_remap(cc, 0, 512)
            emit_vproj(0)
            emit_vproj(1)

            # qk units: st1 fine, then st2+st3 as a double unit with one
            # merged remap of tokens [1024:2048].  Emitted one per 2 slots
            # so every remap lands well before the level that reads it
            # (st1 needed at level 2 / slot 16, st2 at level 4 / slot 32).
            qk_units = [("s1", cc) for cc in range(4)]
            qk_units += [("s23", cc) for cc in range(4)]
            v_units = list(range(2, 16))
            backlog = []

            def pop_pv():
                ph, pj, ppts, psl = backlog.pop(0)
                emit_pv(ph, pj, ppts, psl)

            for j in range(N_J):
                for h in range(8):
                    si = j * 8 + h
                    groups = group_chunks(j)
                    pts_map = {}
                    for gi, g in enumerate(groups):
                        st_t = emit_qk_scores(h, j, g)
                        pts_map[gi] = emit_exp(h, j, g, st_t)
                    emit_mask(h, j, pts_map)
                    backlog.append((h, j, pts_map, (si % 2) * 256))
                    if len(backlog) > 1:
                        pop_pv()
                    if j > 0 and h == 1:
                        for qc in (2 * (j - 1), 2 * (j - 1) + 1):
                            emit_trans(qc)
                            emit_oproj(qc)
                    if si % 2 == 1 and qk_units:
                        kind, cc = qk_units.pop(0)
                        if kind == "s1":
                            emit_qkproj(cc, 1)
                            emit_remap(cc, 512, 1024)
                        else:
                            emit_qkproj(cc, 2)
                            emit_qkproj(cc, 3)
                            emit_remap(cc, 1024, 2048)
                    if h in (2, 5) and v_units:
                        emit_vproj(v_units.pop(0))
            while backlog:
                pop_pv()
            while v_units:
                emit_vproj(v_units.pop(0))
            for qc in (14, 15):
                emit_trans(qc)
                emit_oproj(qc)
    nc.compile()
    return nc


def _get_nc():
    if "nc" not in _CACHE:
        _CACHE["nc"] = _build_nc()
    return _CACHE["nc"]


def _split8(a, scale=1.0):
    hi = (a * scale).astype(ml_dtypes.float8_e4m3)
    lo = (a * scale - hi.astype(np.float32)).astype(ml_dtypes.float8_e5m2)
    return hi, lo


def _wlayout(Wt):
    # Wt: [1024 in, 512 out] -> [128 p, 4 c2, 2 i, 512 out] -> [128, 4096]
    a = Wt.reshape(4, 2, 128, 512).transpose(2, 0, 1, 3)
    return np.ascontiguousarray(a.reshape(128, 4096))


def make_in_maps(x, Wq, bq, Wk, bk, Wv, bv, Wo):
    """Host-side sharding: returns per-core input dicts (core = 2b + hh)."""
    xt = np.ascontiguousarray(
        np.transpose(np.asarray(x, np.float32), (0, 2, 1)))  # [B, H, S]
    xh, xl = _split8(xt)
    tri = np.triu(np.ones((128, 128), np.float32)).astype(ml_dtypes.bfloat16)

    packs = []
    for hh in range(2):
        r = slice(hh * C, (hh + 1) * C)
        wqh_, wql_ = _split8(_wlayout(np.asarray(Wq, np.float32)[r, :].T), WS)
        wkh_, wkl_ = _split8(_wlayout(np.asarray(Wk, np.float32)[r, :].T), WS)
        wvh_, wvl_ = _split8(_wlayout(np.asarray(Wv, np.float32)[r, :].T), WS)
        wo_ = np.asarray(Wo, np.float32)[:, r].T  # [512 in, 1024 out]
        wo_ = np.ascontiguousarray(
            wo_.reshape(4, 128, 1024).transpose(1, 0, 2).reshape(128, 4096)
        ).astype(ml_dtypes.bfloat16)
        bq_ = np.ascontiguousarray(
            np.asarray(bq, np.float32)[r].reshape(4, 128).T)
        bk_ = np.ascontiguousarray(
            np.asarray(bk, np.float32)[r].reshape(4, 128).T)
        packs.append({"wqh": wqh_, "wql": wql_, "wkh": wkh_, "wkl": wkl_,
                      "wvh": wvh_, "wvl": wvl_, "wo": wo_,
                      "bq": bq_, "bk": bk_})

    in_maps = []
    for c in range(NCORES):
        b, hh = c // 2, c % 2
        m = {"xh": xh[b], "xl": xl[b], "tri": tri}
        m.update(packs[hh])
        in_maps.append(m)
    return in_maps


def run_cores(in_maps):
    nc = _get_nc()
    res = run_bass_kernel_spmd(nc, in_maps, core_ids=list(range(NCORES)))
    return [r["out"] for r in res.results]


def host_combine(partials, bo, bv, Wo):
    out = np.zeros((B, S, H), np.float32)
    for b in range(B):
        out[b] = (np.asarray(partials[2 * b]).astype(np.float32)
                  + np.asarray(partials[2 * b + 1]).astype(np.float32))
    bias = (np.asarray(bo, np.float32)
            + np.asarray(bv, np.float32) @ np.asarray(Wo, np.float32).T)
    return out + bias[None, None, :]


def kernel(x, mask, Wq, bq, Wk, bk, Wv, bv, Wo, bo):
    in_maps = make_in_maps(x, Wq, bq, Wk, bk, Wv, bv, Wo)
    partials = run_cores(in_maps)
    return host_combine(partials, bo, bv, Wo).astype(np.float32)


# revision 14
# speedup vs baseline: 1.0563x; 1.0563x over previous
"""Multi-head causal attention (B=4, S=2048, H=1024, NH=16) on 8 trn2 cores.

Hybrid sharding: core = (batch b, head-half hh) -> 1 batch x 8 heads per
core.  fp8 DoubleRow matmuls carry the projections (hi/lo e4m3+e5m2
compensation: Q/K 2 passes, V 3) and the Q*K score matmuls (e4m3 stores);
P@V / output projection run bf16.  Scores are computed transposed
S^T[k,q] so softmax denominators fall out of a ones-column in the P@V
accumulation and normalization is a per-partition scalar multiply.  The
ctx^T -> ctx transpose uses the DMA xbar (no PE/PSUM involved), and the
output projection contracts all 512 local channels so each core emits a
[2048,1024] partial for its batch; the host sums the 2 partials per
batch and folds in bo + bv@Wo^T.  Slot order is j-outer/h-inner so
transposes + output projection pipeline level-by-level."""
import numpy as np
import ml_dtypes

import concourse.bacc as bacc
import concourse.tile as tile
from concourse import mybir
from concourse.bass_utils import run_bass_kernel_spmd

F32 = mybir.dt.float32
BF16 = mybir.dt.bfloat16
E4 = mybir.dt.float8e4
E5 = mybir.dt.float8e5
AF = mybir.ActivationFunctionType
DR = mybir.MatmulPerfMode.DoubleRow
MULT = mybir.AluOpType.mult
ADD = mybir.AluOpType.add

B, S, H, NH = 4, 2048, 1024, 16
HD = H // NH            # 64
NCORES = 8
HPC = 8                 # heads per core
C = HPC * HD            # 512 channels per core
SCALE = 1.0 / np.sqrt(HD)
WS = 32.0               # weight pre-scale (keeps fp8 hi in normal range)

N_J = S // 256          # 8 j-levels of 256 q each
QK_PASSES = 2
V_PASSES = 3

_CACHE = {}


def _build_nc():
    nc = bacc.Bacc(name="mha_v2")
    xh_d = nc.dram_tensor("xh", [H, S], E4, kind="ExternalInput")
    xl_d = nc.dram_tensor("xl", [H, S], E5, kind="ExternalInput")
    wqh_d = nc.dram_tensor("wqh", [128, 4096], E4, kind="ExternalInput")
    wql_d = nc.dram_tensor("wql", [128, 4096], E5, kind="ExternalInput")
    wkh_d = nc.dram_tensor("wkh", [128, 4096], E4, kind="ExternalInput")
    wkl_d = nc.dram_tensor("wkl", [128, 4096], E5, kind="ExternalInput")
    wvh_d = nc.dram_tensor("wvh", [128, 4096], E4, kind="ExternalInput")
    wvl_d = nc.dram_tensor("wvl", [128, 4096], E5, kind="ExternalInput")
    wo_d = nc.dram_tensor("wo", [128, 4096], BF16, kind="ExternalInput")
    bq_d = nc.dram_tensor("bq", [128, 4], F32, kind="ExternalInput")
    bk_d = nc.dram_tensor("bk", [128, 4], F32, kind="ExternalInput")
    tri_d = nc.dram_tensor("tri", [128, 128], BF16, kind="ExternalInput")
    out_d = nc.dram_tensor("out", [S, H], BF16, kind="ExternalOutput")

    with tile.TileContext(nc) as tc:
        with (
            tc.tile_pool(name="const", bufs=1) as cp,
            tc.tile_pool(name="xs", bufs=1) as xp,
            tc.tile_pool(name="qk", bufs=1) as qp,
            tc.tile_pool(name="vn", bufs=1) as vp,
            tc.tile_pool(name="pts", bufs=10) as pp,
            tc.tile_pool(name="ctx", bufs=1) as ctp,
            tc.tile_pool(name="small", bufs=8) as sp,
            tc.tile_pool(name="osb", bufs=3) as op_,
            tc.tile_pool(name="stp", bufs=2, space="PSUM") as stp,
            tc.tile_pool(name="accp", bufs=1, space="PSUM") as accp,
            tc.tile_pool(name="mixp", bufs=3, space="PSUM") as mixp,
        ):
            # ---- constants ----
            wqh_s = cp.tile([128, 4096], E4)
            wql_s = cp.tile([128, 4096], E5)
            wkh_s = cp.tile([128, 4096], E4)
            wkl_s = cp.tile([128, 4096], E5)
            wvh_s = cp.tile([128, 4096], E4)
            wvl_s = cp.tile([128, 4096], E5)
            wo_s = cp.tile([128, 4096], BF16)
            bq_s = cp.tile([128, 4], F32)
            bk_s = cp.tile([128, 4], F32)
            tri_s = cp.tile([128, 128], BF16)
            for s_, d_ in ((wqh_s, wqh_d), (wql_s, wql_d), (bq_s, bq_d),
                           (wkh_s, wkh_d), (wkl_s, wkl_d), (bk_s, bk_d),
                           (tri_s, tri_d)):
                nc.sync.dma_start(s_[:], d_.ap())

            # x tiles (one batch): [128 p, 8 c, 2048 t], ch = c*128 + p
            xh_s = xp.tile([128, 16384], E4)
            xl_s = xp.tile([128, 16384], E5)

            def emit_xload(s_, d_, q):
                qsl = slice(q * 512, (q + 1) * 512)
                nc.sync.dma_start(
                    s_.rearrange("p (c t) -> p c t", c=8)[:, :, qsl],
                    d_.ap()[:, qsl].rearrange("(c p) t -> p c t", p=128))

            emit_xload(xh_s, xh_d, 0)
            emit_xload(xl_s, xl_d, 0)
            for s_, d_ in ((wvh_s, wvh_d), (wvl_s, wvl_d)):
                nc.sync.dma_start(s_[:], d_.ap())
            for q in range(1, 4):
                emit_xload(xh_s, xh_d, q)
            for q in range(1, 4):
                emit_xload(xl_s, xl_d, q)
            nc.sync.dma_start(wo_s[:], wo_d.ap())

            # weight views [p, c2, i, out512]
            wv_ = {
                "qh": wqh_s.rearrange("p (c i o) -> p c i o", c=4, i=2),
                "ql": wql_s.rearrange("p (c i o) -> p c i o", c=4, i=2),
                "kh": wkh_s.rearrange("p (c i o) -> p c i o", c=4, i=2),
                "kl": wkl_s.rearrange("p (c i o) -> p c i o", c=4, i=2),
                "vh": wvh_s.rearrange("p (c i o) -> p c i o", c=4, i=2),
                "vl": wvl_s.rearrange("p (c i o) -> p c i o", c=4, i=2),
            }
            wo_v = wo_s.rearrange("p (cc o) -> p cc o", cc=4)
            xh_v = xh_s.rearrange("p (c i t) -> p c i t", c=4, i=2)
            xl_v = xl_s.rearrange("p (c i t) -> p c i t", c=4, i=2)

            # q/k channel-major tiles per cc chunk (2 heads each)
            qt = [qp.tile([128, 2048], E4, name=f"qt{cc}") for cc in range(4)]
            kt = [qp.tile([128, 2048], E4, name=f"kt{cc}") for cc in range(4)]
            # DR-packed q/k: [64 (2h x 32), 2 i, 2048 t] per head-pair cc
            qdr = [qp.tile([64, 4096], E4, name=f"qdr{g}") for g in range(4)]
            kdr = [qp.tile([64, 4096], E4, name=f"kdr{g}") for g in range(4)]
            # v (+ones col): [128 k-part, 8 h, 16 kc, 65]
            vn = vp.tile([128, 8320], BF16)
            vn4 = vn.rearrange("p (h c e) -> p h c e", h=8, e=65)
            nc.vector.memset(vn4[:, :, :, 64], 1.0)
            # ctx^T [128 tok, 16 qc, 8 h, 64 d]; ctx [128 chp, 4 cc, 2048 t]
            ctxT = ctp.tile([128, 8192], BF16)
            ctx = ctp.tile([128, 8192], BF16)
            ctx3 = ctx.rearrange("p (cc t) -> p cc t", cc=4)
            acc = accp.tile([128, 512], F32)
            accv = acc.rearrange("p (s q e) -> p s q e", s=2, q=2)

            def emit_qkproj(cc, st, which):
                tsl = slice(st * 512, (st + 1) * 512)
                wh, wl, bias, dst = (("qh", "ql", bq_s, qt) if which == "q"
                                     else ("kh", "kl", bk_s, kt))
                pmm = mixp.tile([128, 512], F32, tag="mix",
                                name=f"pp{wh}{cc}_{st}")
                first = True
                for w_v in (wv_[wh], wv_[wl])[:QK_PASSES]:
                    for c2 in range(4):
                        nc.tensor.matmul(
                            pmm[:],
                            w_v[:, c2, :, cc * 128:(cc + 1) * 128],
                            xh_v[:, c2, :, tsl],
                            start=first,
                            stop=(w_v is wv_[wl] and c2 == 3),
                            perf_mode=DR)
                        first = False
                nc.vector.tensor_scalar(dst[cc][:, tsl], pmm[:], 1.0 / WS,
                                        bias[:, cc:cc + 1], op0=MULT,
                                        op1=ADD)

            def emit_remap(cc, t0, t1):
                tsl = slice(t0, t1)
                for src_l, dr in ((qt, qdr), (kt, kdr)):
                    drv = dr[cc].rearrange("p (i t) -> p i t", i=2)
                    for hl in range(2):
                        p0 = hl * 32
                        for i in range(2):
                            nc.sync.dma_start(
                                drv[p0:p0 + 32, i, tsl],
                                src_l[cc][hl * 64 + i * 32:
                                          hl * 64 + i * 32 + 32, tsl])

            def emit_vproj(c):
                # flipped V: psum [128 tok, 512 ch] for one 128-token chunk
                tsl = slice(c * 128, (c + 1) * 128)
                vm = mixp.tile([128, 512], F32, tag="mix", name=f"vp{c}")
                passes = [("vh", xh_v), ("vl", xh_v)]
                if V_PASSES >= 3:
                    passes.append(("vh", xl_v))
                first = True
                for wn, x_v in passes:
                    for c2 in range(4):
                        nc.tensor.matmul(
                            vm[:], x_v[:, c2, :, tsl], wv_[wn][:, c2],
                            start=first,
                            stop=(wn == passes[-1][0] and x_v is passes[-1][1]
                                  and c2 == 3),
                            perf_mode=DR)
                        first = False
                nc.vector.tensor_scalar(
                    vn4[:, :, c, 0:64],
                    vm.rearrange("p (h d) -> p h d", h=8), 1.0 / WS, None,
                    op0=MULT)

            def group_chunks(j):
                nkc = 2 * (j + 1)
                groups = []
                for g0 in range(0, nkc, 4):
                    cs = list(range(g0, min(g0 + 4, nkc)))
                    pos, w = [], []
                    for c in cs:
                        pos.append((c - g0) * 256)
                        w.append(128 if c == nkc - 1 else 256)
                    groups.append((cs, w, pos, pos[-1] + w[-1]))
                return groups

            def emit_qk_scores(h, j, g):
                qv = qdr[h // 2].rearrange("p (i t) -> p i t", i=2)
                kv = kdr[h // 2].rearrange("p (i t) -> p i t", i=2)
                hsl = slice((h % 2) * 32, (h % 2) * 32 + 32)
                cs, ws, poss, tw = g
                st = stp.tile([128, 1024], F32, tag="st",
                              name=f"st{h}_{j}_{cs[0]}")
                nkc = 2 * (j + 1)
                for c, w, pos in zip(cs, ws, poss):
                    q0 = j * 256 + (128 if c == nkc - 1 else 0)
                    nc.tensor.matmul(
                        st[:, pos:pos + w],
                        kv[hsl, :, c * 128:(c + 1) * 128],
                        qv[hsl, :, q0:q0 + w],
                        start=True, stop=True, perf_mode=DR)
                return st

            def emit_exp(h, j, g, st):
                cs, ws, poss, tw = g
                pt = pp.tile([128, 1024], BF16, tag="pt",
                             name=f"pt{h}_{j}_{cs[0]}")
                nc.scalar.activation(pt[:, 0:tw], st[:, 0:tw], AF.Exp,
                                     scale=float(SCALE))
                return pt

            def emit_mask(h, j, pts_map):
                nkc = 2 * (j + 1)
                for c in (nkc - 2, nkc - 1):
                    pt = pts_map[c // 4]
                    pos = (c % 4) * 256
                    nc.gpsimd.tensor_mul(pt[:, pos:pos + 128],
                                         pt[:, pos:pos + 128], tri_s[:])

            def emit_pv_qb(h, j, pts_map, sl, qb):
                nkc = 2 * (j + 1)
                qc = 2 * j + qb
                for c in range(qc + 1):
                    pt = pts_map[c // 4]
                    pos = (c % 4) * 256 + (qb * 128 if c < nkc - 1 else 0)
                    nc.tensor.matmul(
                        acc[:, sl + qb * 128: sl + qb * 128 + 65],
                        pt[:, pos:pos + 128],
                        vn[:, h * 1040 + c * 65: h * 1040 + (c + 1) * 65],
                        start=(c == 0), stop=(c == qc),
                        skip_group_check=True)

            def emit_pv_fin(h, j, sl):
                den = sp.tile([128, 2], F32, tag="den", name=f"dn{h}_{j}")
                s2 = sl // 256
                nc.vector.reciprocal(den[:], accv[:, s2, :, 64])
                for qb in range(2):
                    qc = 2 * j + qb
                    nc.vector.tensor_scalar(
                        ctxT[:, qc * 512 + h * 64: qc * 512 + (h + 1) * 64],
                        acc[:, sl + qb * 128: sl + qb * 128 + 64],
                        den[:, qb:qb + 1], None, op0=MULT)

            def emit_trans(qc):
                nc.sync.dma_start_transpose(
                    ctx3[:, :, qc * 128:(qc + 1) * 128],
                    ctxT[:, qc * 512:(qc + 1) * 512])

            osb_map = {}

            def emit_om(qc, half):
                if qc not in osb_map:
                    osb_map[qc] = op_.tile([128, 1024], BF16, tag="osb",
                                           name=f"ob{qc}")
                om = mixp.tile([128, 512], F32, tag="mix",
                               name=f"om{qc}_{half}")
                for cc in range(4):
                    nc.tensor.matmul(
                        om[:],
                        ctx3[:, cc, qc * 128:(qc + 1) * 128],
                        wo_v[:, cc, half * 512:(half + 1) * 512],
                        start=(cc == 0), stop=(cc == 3))
                nc.vector.tensor_copy(
                    osb_map[qc][:, half * 512:(half + 1) * 512], om[:])

            def emit_outdma(qc):
                nc.sync.dma_start(
                    out_d.ap()[qc * 128:(qc + 1) * 128, :], osb_map[qc][:])

            # ---------------- emission schedule ----------------
            # PE work other than the per-slot score matmuls is queued as
            # "filler" closures, drained one per score group so the PE
            # instruction stream never has a multi-us block that would
            # starve the ACT engine (scores feed exp, the bottleneck).
            import collections as _c
            pe_fill = _c.deque()

            def drain(n):
                while n > 0 and pe_fill:
                    pe_fill.popleft()()
                    n -= 1

            # prologue: QK projections + remaps for tokens 0:512, V chunks
            # 0-1 (everything level 0/1 needs); cc0 first so slot (0,0)
            # starts as early as possible.
            emit_qkproj(0, 0, "q")
            emit_qkproj(0, 0, "k")
            emit_remap(0, 0, 512)
            for cc in range(1, 4):
                pe_fill.append(lambda cc=cc: emit_qkproj(cc, 0, "q"))
                pe_fill.append(lambda cc=cc: emit_qkproj(cc, 0, "k"))
                pe_fill.append(lambda cc=cc: emit_remap(cc, 0, 512))
            pe_fill.append(lambda: emit_vproj(0))
            pe_fill.append(lambda: emit_vproj(1))
            drain(4)

            # remaining QK proj units: st1 fine-grained remap, st2+st3 with
            # a merged remap of tokens [1024:2048]; pushed one per 2 slots.
            qk_units = [("s1", cc) for cc in range(4)]
            qk_units += [("s23", cc) for cc in range(4)]
            v_units = list(range(2, 16))

            def push_qk(kind, cc):
                if kind == "s1":
                    pe_fill.append(lambda: emit_qkproj(cc, 1, "q"))
                    pe_fill.append(lambda: emit_qkproj(cc, 1, "k"))
                    pe_fill.append(lambda: emit_remap(cc, 512, 1024))
                else:
                    pe_fill.append(lambda: emit_qkproj(cc, 2, "q"))
                    pe_fill.append(lambda: emit_qkproj(cc, 2, "k"))
                    pe_fill.append(lambda: emit_qkproj(cc, 3, "q"))
                    pe_fill.append(lambda: emit_qkproj(cc, 3, "k"))
                    pe_fill.append(lambda: emit_remap(cc, 1024, 2048))

            for j in range(N_J):
                for h in range(8):
                    si = j * 8 + h
                    groups = group_chunks(j)
                    pts_map = {}
                    for gi, g in enumerate(groups):
                        st_t = emit_qk_scores(h, j, g)
                        pts_map[gi] = emit_exp(h, j, g, st_t)
                        drain(1)
                    emit_mask(h, j, pts_map)
                    # flush stragglers so pv/norm of slot s-1 lands in slot s
                    drain(len(pe_fill) - 3)
                    sl = (si % 2) * 256
                    pe_fill.append(
                        lambda h=h, j=j, p=pts_map, sl=sl:
                            emit_pv_qb(h, j, p, sl, 0))
                    pe_fill.append(
                        lambda h=h, j=j, p=pts_map, sl=sl:
                            emit_pv_qb(h, j, p, sl, 1))
                    pe_fill.append(
                        lambda h=h, j=j, sl=sl: emit_pv_fin(h, j, sl))
                    if j > 0 and h == 1:
                        # level j-1 fully normalized (flushed above)
                        for qc in (2 * (j - 1), 2 * (j - 1) + 1):
                            emit_trans(qc)
                            for half in range(2):
                                pe_fill.append(
                                    lambda qc=qc, half=half:
                                        emit_om(qc, half))
                            pe_fill.append(lambda qc=qc: emit_outdma(qc))
                    if si % 2 == 1 and qk_units:
                        push_qk(*qk_units.pop(0))
                    if h in (2, 5) and v_units:
                        c = v_units.pop(0)
                        pe_fill.append(lambda c=c: emit_vproj(c))
            drain(len(pe_fill))
            for qc in (14, 15):
                emit_trans(qc)
                emit_om(qc, 0)
                emit_om(qc, 1)
                emit_outdma(qc)
    nc.compile()
    return nc


def _get_nc():
    if "nc" not in _CACHE:
        _CACHE["nc"] = _build_nc()
    return _CACHE["nc"]


def _split8(a, scale=1.0):
    hi = (a * scale).astype(ml_dtypes.float8_e4m3)
    lo = (a * scale - hi.astype(np.float32)).astype(ml_dtypes.float8_e5m2)
    return hi, lo


def _wlayout(Wt):
    # Wt: [1024 in, 512 out] -> [128 p, 4 c2, 2 i, 512 out] -> [128, 4096]
    a = Wt.reshape(4, 2, 128, 512).transpose(2, 0, 1, 3)
    return np.ascontiguousarray(a.reshape(128, 4096))


def make_in_maps(x, Wq, bq, Wk, bk, Wv, bv, Wo):
    """Host-side sharding: returns per-core input dicts (core = 2b + hh)."""
    xt = np.ascontiguousarray(
        np.transpose(np.asarray(x, np.float32), (0, 2, 1)))  # [B, H, S]
    xh, xl = _split8(xt)
    tri = np.triu(np.ones((128, 128), np.float32)).astype(ml_dtypes.bfloat16)

    packs = []
    for hh in range(2):
        r = slice(hh * C, (hh + 1) * C)
        wqh_, wql_ = _split8(_wlayout(np.asarray(Wq, np.float32)[r, :].T), WS)
        wkh_, wkl_ = _split8(_wlayout(np.asarray(Wk, np.float32)[r, :].T), WS)
        wvh_, wvl_ = _split8(_wlayout(np.asarray(Wv, np.float32)[r, :].T), WS)
        wo_ = np.asarray(Wo, np.float32)[:, r].T  # [512 in, 1024 out]
        wo_ = np.ascontiguousarray(
            wo_.reshape(4, 128, 1024).transpose(1, 0, 2).reshape(128, 4096)
        ).astype(ml_dtypes.bfloat16)
        bq_ = np.ascontiguousarray(
            np.asarray(bq, np.float32)[r].reshape(4, 128).T)
        bk_ = np.ascontiguousarray(
            np.asarray(bk, np.float32)[r].reshape(4, 128).T)
        packs.append({"wqh": wqh_, "wql": wql_, "wkh": wkh_, "wkl": wkl_,
                      "wvh": wvh_, "wvl": wvl_, "wo": wo_,
                      "bq": bq_, "bk": bk_})

    in_maps = []
    for c in range(NCORES):
        b, hh = c // 2, c % 2
        m = {"xh": xh[b], "xl": xl[b], "tri": tri}
        m.update(packs[hh])
        in_maps.append(m)
    return in_maps


def run_cores(in_maps):
    nc = _get_nc()
    res = run_bass_kernel_spmd(nc, in_maps, core_ids=list(range(NCORES)))
    return [r["out"] for r in res.results]


def host_combine(partials, bo, bv, Wo):
    out = np.zeros((B, S, H), np.float32)
    for b in range(B):
        out[b] = (np.asarray(partials[2 * b]).astype(np.float32)
                  + np.asarray(partials[2 * b + 1]).astype(np.float32))
    bias = (np.asarray(bo, np.float32)
            + np.asarray(bv, np.float32) @ np.asarray(Wo, np.float32).T)
    return out + bias[None, None, :]


def kernel(x, mask, Wq, bq, Wk, bk, Wv, bv, Wo, bo):
    in_maps = make_in_maps(x, Wq, bq, Wk, bk, Wv, bv, Wo)
    partials = run_cores(in_maps)
    return host_combine(partials, bo, bv, Wo).astype(np.float32)


# revision 17
# speedup vs baseline: 1.1086x; 1.0495x over previous
"""Multi-head causal attention (B=4, S=2048, H=1024, NH=16) on 8 trn2 cores.

Hybrid sharding: core = (batch b, head-half hh) -> 1 batch x 8 heads per
core.  fp8 DoubleRow matmuls carry the projections (hi/lo e4m3+e5m2
compensation: Q/K 2 passes, V 3) and the Q*K score matmuls (e4m3 stores);
P@V / output projection run bf16.  Scores are computed transposed
S^T[k,q] so softmax denominators fall out of a ones-column in the P@V
accumulation and normalization is a per-partition scalar multiply.  The
ctx^T -> ctx transpose uses the DMA xbar (no PE/PSUM involved), and the
output projection contracts all 512 local channels so each core emits a
[2048,1024] partial for its batch; the host sums the 2 partials per
batch and folds in bo + bv@Wo^T.  Slot order is j-outer/h-inner so
transposes + output projection pipeline level-by-level."""
import numpy as np
import ml_dtypes

import concourse.bacc as bacc
import concourse.tile as tile
from concourse import mybir
from concourse.bass_utils import run_bass_kernel_spmd

F32 = mybir.dt.float32
BF16 = mybir.dt.bfloat16
E4 = mybir.dt.float8e4
E5 = mybir.dt.float8e5
AF = mybir.ActivationFunctionType
DR = mybir.MatmulPerfMode.DoubleRow
MULT = mybir.AluOpType.mult
ADD = mybir.AluOpType.add

B, S, H, NH = 4, 2048, 1024, 16
HD = H // NH            # 64
NCORES = 8
HPC = 8                 # heads per core
C = HPC * HD            # 512 channels per core
SCALE = 1.0 / np.sqrt(HD)
WS = 32.0               # weight pre-scale (keeps fp8 hi in normal range)

N_J = S // 256          # 8 j-levels of 256 q each
QK_PASSES = 2
V_PASSES = 3

_CACHE = {}


def _build_nc():
    nc = bacc.Bacc(name="mha_v2")
    xh_d = nc.dram_tensor("xh", [H, S], E4, kind="ExternalInput")
    xl_d = nc.dram_tensor("xl", [H, S], E5, kind="ExternalInput")
    wqh_d = nc.dram_tensor("wqh", [128, 4096], E4, kind="ExternalInput")
    wql_d = nc.dram_tensor("wql", [128, 4096], E5, kind="ExternalInput")
    wkh_d = nc.dram_tensor("wkh", [128, 4096], E4, kind="ExternalInput")
    wkl_d = nc.dram_tensor("wkl", [128, 4096], E5, kind="ExternalInput")
    wvh_d = nc.dram_tensor("wvh", [128, 4096], E4, kind="ExternalInput")
    wvl_d = nc.dram_tensor("wvl", [128, 4096], E5, kind="ExternalInput")
    wo_d = nc.dram_tensor("wo", [128, 4096], BF16, kind="ExternalInput")
    bq_d = nc.dram_tensor("bq", [128, 4], F32, kind="ExternalInput")
    bk_d = nc.dram_tensor("bk", [128, 4], F32, kind="ExternalInput")
    tri_d = nc.dram_tensor("tri", [128, 128], BF16, kind="ExternalInput")
    out_d = nc.dram_tensor("out", [S, H], BF16, kind="ExternalOutput")

    with tile.TileContext(nc) as tc:
        with (
            tc.tile_pool(name="const", bufs=1) as cp,
            tc.tile_pool(name="xs", bufs=1) as xp,
            tc.tile_pool(name="qk", bufs=1) as qp,
            tc.tile_pool(name="vn", bufs=1) as vp,
            tc.tile_pool(name="pts", bufs=10) as pp,
            tc.tile_pool(name="ctx", bufs=1) as ctp,
            tc.tile_pool(name="small", bufs=8) as sp,
            tc.tile_pool(name="osb", bufs=3) as op_,
            tc.tile_pool(name="stp", bufs=2, space="PSUM") as stp,
            tc.tile_pool(name="accp", bufs=1, space="PSUM") as accp,
            tc.tile_pool(name="mixp", bufs=3, space="PSUM") as mixp,
        ):
            # ---- constants ----
            wqh_s = cp.tile([128, 4096], E4)
            wql_s = cp.tile([128, 4096], E5)
            wkh_s = cp.tile([128, 4096], E4)
            wkl_s = cp.tile([128, 4096], E5)
            wvh_s = cp.tile([128, 4096], E4)
            wvl_s = cp.tile([128, 4096], E5)
            wo_s = cp.tile([128, 4096], BF16)
            bq_s = cp.tile([128, 4], F32)
            bk_s = cp.tile([128, 4], F32)
            tri_s = cp.tile([128, 128], BF16)
            for s_, d_ in ((wqh_s, wqh_d), (wql_s, wql_d), (bq_s, bq_d),
                           (wkh_s, wkh_d), (wkl_s, wkl_d), (bk_s, bk_d),
                           (tri_s, tri_d)):
                nc.sync.dma_start(s_[:], d_.ap())

            # x tiles (one batch): [128 p, 8 c, 2048 t], ch = c*128 + p
            xh_s = xp.tile([128, 16384], E4)
            xl_s = xp.tile([128, 16384], E5)

            def emit_xload(s_, d_, q):
                qsl = slice(q * 512, (q + 1) * 512)
                nc.sync.dma_start(
                    s_.rearrange("p (c t) -> p c t", c=8)[:, :, qsl],
                    d_.ap()[:, qsl].rearrange("(c p) t -> p c t", p=128))

            emit_xload(xh_s, xh_d, 0)
            emit_xload(xl_s, xl_d, 0)
            for s_, d_ in ((wvh_s, wvh_d), (wvl_s, wvl_d)):
                nc.sync.dma_start(s_[:], d_.ap())
            for q in range(1, 4):
                emit_xload(xh_s, xh_d, q)
            for q in range(1, 4):
                emit_xload(xl_s, xl_d, q)
            nc.sync.dma_start(wo_s[:], wo_d.ap())

            # weight views [p, c2, i, out512]
            wv_ = {
                "qh": wqh_s.rearrange("p (c i o) -> p c i o", c=4, i=2),
                "ql": wql_s.rearrange("p (c i o) -> p c i o", c=4, i=2),
                "kh": wkh_s.rearrange("p (c i o) -> p c i o", c=4, i=2),
                "kl": wkl_s.rearrange("p (c i o) -> p c i o", c=4, i=2),
                "vh": wvh_s.rearrange("p (c i o) -> p c i o", c=4, i=2),
                "vl": wvl_s.rearrange("p (c i o) -> p c i o", c=4, i=2),
            }
            wo_v = wo_s.rearrange("p (cc o) -> p cc o", cc=4)
            xh_v = xh_s.rearrange("p (c i t) -> p c i t", c=4, i=2)
            xl_v = xl_s.rearrange("p (c i t) -> p c i t", c=4, i=2)

            # q/k channel-major tiles per cc chunk (2 heads each)
            qt = [qp.tile([128, 2048], E4, name=f"qt{cc}") for cc in range(4)]
            kt = [qp.tile([128, 2048], E4, name=f"kt{cc}") for cc in range(4)]
            # DR-packed q/k: [64 (2h x 32), 2 i, 2048 t] per head-pair cc
            qdr = [qp.tile([64, 4096], E4, name=f"qdr{g}") for g in range(4)]
            kdr = [qp.tile([64, 4096], E4, name=f"kdr{g}") for g in range(4)]
            # v (+ones col): [128 k-part, 8 h, 16 kc, 65]
            vn = vp.tile([128, 8320], BF16)
            vn4 = vn.rearrange("p (h c e) -> p h c e", h=8, e=65)
            nc.vector.memset(vn4[:, :, :, 64], 1.0)
            # ctx^T [128 tok, 16 qc, 8 h, 64 d]; ctx [128 chp, 4 cc, 2048 t]
            ctxT = ctp.tile([128, 8192], BF16)
            ctx = ctp.tile([128, 8192], BF16)
            ctx3 = ctx.rearrange("p (cc t) -> p cc t", cc=4)
            acc = accp.tile([128, 512], F32)
            accv = acc.rearrange("p (s q e) -> p s q e", s=2, q=2)

            def emit_qkproj(cc, st, which):
                tsl = slice(st * 512, (st + 1) * 512)
                wh, wl, bias, dst = (("qh", "ql", bq_s, qt) if which == "q"
                                     else ("kh", "kl", bk_s, kt))
                pmm = mixp.tile([128, 512], F32, tag="mix",
                                name=f"pp{wh}{cc}_{st}")
                first = True
                for w_v in (wv_[wh], wv_[wl])[:QK_PASSES]:
                    for c2 in range(4):
                        nc.tensor.matmul(
                            pmm[:],
                            w_v[:, c2, :, cc * 128:(cc + 1) * 128],
                            xh_v[:, c2, :, tsl],
                            start=first,
                            stop=(w_v is wv_[wl] and c2 == 3),
                            perf_mode=DR)
                        first = False
                nc.vector.tensor_scalar(dst[cc][:, tsl], pmm[:], 1.0 / WS,
                                        bias[:, cc:cc + 1], op0=MULT,
                                        op1=ADD)

            remap_rr = [0]

            def emit_remap(cc, t0, t1):
                # alternate between the SP HWDGE queue and the Pool SWDGE
                # queue: 96 remap DMAs through one serialized DGE device
                # would wall the first ~100us of the kernel.
                tsl = slice(t0, t1)
                for src_l, dr in ((qt, qdr), (kt, kdr)):
                    drv = dr[cc].rearrange("p (i t) -> p i t", i=2)
                    for hl in range(2):
                        p0 = hl * 32
                        for i in range(2):
                            eng = (nc.sync, nc.gpsimd)[remap_rr[0] % 2]
                            remap_rr[0] += 1
                            eng.dma_start(
                                drv[p0:p0 + 32, i, tsl],
                                src_l[cc][hl * 64 + i * 32:
                                          hl * 64 + i * 32 + 32, tsl])

            def emit_vproj(c):
                # flipped V: psum [128 tok, 512 ch] for one 128-token chunk
                tsl = slice(c * 128, (c + 1) * 128)
                vm = mixp.tile([128, 512], F32, tag="mix", name=f"vp{c}")
                passes = [("vh", xh_v), ("vl", xh_v)]
                if V_PASSES >= 3:
                    passes.append(("vh", xl_v))
                first = True
                for wn, x_v in passes:
                    for c2 in range(4):
                        nc.tensor.matmul(
                            vm[:], x_v[:, c2, :, tsl], wv_[wn][:, c2],
                            start=first,
                            stop=(wn == passes[-1][0] and x_v is passes[-1][1]
                                  and c2 == 3),
                            perf_mode=DR)
                        first = False
                nc.vector.tensor_scalar(
                    vn4[:, :, c, 0:64],
                    vm.rearrange("p (h d) -> p h d", h=8), 1.0 / WS, None,
                    op0=MULT)

            def group_chunks(j):
                nkc = 2 * (j + 1)
                groups = []
                for g0 in range(0, nkc, 4):
                    cs = list(range(g0, min(g0 + 4, nkc)))
                    pos, w = [], []
                    for c in cs:
                        pos.append((c - g0) * 256)
                        w.append(128 if c == nkc - 1 else 256)
                    groups.append((cs, w, pos, pos[-1] + w[-1]))
                return groups

            def emit_qk_scores(h, j, g):
                qv = qdr[h // 2].rearrange("p (i t) -> p i t", i=2)
                kv = kdr[h // 2].rearrange("p (i t) -> p i t", i=2)
                hsl = slice((h % 2) * 32, (h % 2) * 32 + 32)
                cs, ws, poss, tw = g
                st = stp.tile([128, 1024], F32, tag="st",
                              name=f"st{h}_{j}_{cs[0]}")
                nkc = 2 * (j + 1)
                for c, w, pos in zip(cs, ws, poss):
                    q0 = j * 256 + (128 if c == nkc - 1 else 0)
                    nc.tensor.matmul(
                        st[:, pos:pos + w],
                        kv[hsl, :, c * 128:(c + 1) * 128],
                        qv[hsl, :, q0:q0 + w],
                        start=True, stop=True, perf_mode=DR)
                return st

            def emit_exp(h, j, g, st):
                cs, ws, poss, tw = g
                pt = pp.tile([128, 1024], BF16, tag="pt",
                             name=f"pt{h}_{j}_{cs[0]}")
                nc.scalar.activation(pt[:, 0:tw], st[:, 0:tw], AF.Exp,
                                     scale=float(SCALE))
                return pt

            def emit_mask(h, j, pts_map):
                nkc = 2 * (j + 1)
                for c in (nkc - 2, nkc - 1):
                    pt = pts_map[c // 4]
                    pos = (c % 4) * 256
                    nc.gpsimd.tensor_mul(pt[:, pos:pos + 128],
                                         pt[:, pos:pos + 128], tri_s[:])

            def emit_pv_qb(h, j, pts_map, sl, qb):
                nkc = 2 * (j + 1)
                qc = 2 * j + qb
                for c in range(qc + 1):
                    pt = pts_map[c // 4]
                    pos = (c % 4) * 256 + (qb * 128 if c < nkc - 1 else 0)
                    nc.tensor.matmul(
                        acc[:, sl + qb * 128: sl + qb * 128 + 65],
                        pt[:, pos:pos + 128],
                        vn[:, h * 1040 + c * 65: h * 1040 + (c + 1) * 65],
                        start=(c == 0), stop=(c == qc),
                        skip_group_check=True)

            def emit_pv_fin(h, j, sl):
                den = sp.tile([128, 2], F32, tag="den", name=f"dn{h}_{j}")
                s2 = sl // 256
                nc.vector.reciprocal(den[:], accv[:, s2, :, 64])
                for qb in range(2):
                    qc = 2 * j + qb
                    nc.vector.tensor_scalar(
                        ctxT[:, qc * 512 + h * 64: qc * 512 + (h + 1) * 64],
                        acc[:, sl + qb * 128: sl + qb * 128 + 64],
                        den[:, qb:qb + 1], None, op0=MULT)

            def emit_trans(qc):
                nc.sync.dma_start_transpose(
                    ctx3[:, :, qc * 128:(qc + 1) * 128],
                    ctxT[:, qc * 512:(qc + 1) * 512])

            osb_map = {}

            def emit_om(qc, half):
                if qc not in osb_map:
                    osb_map[qc] = op_.tile([128, 1024], BF16, tag="osb",
                                           name=f"ob{qc}")
                om = mixp.tile([128, 512], F32, tag="mix",
                               name=f"om{qc}_{half}")
                for cc in range(4):
                    nc.tensor.matmul(
                        om[:],
                        ctx3[:, cc, qc * 128:(qc + 1) * 128],
                        wo_v[:, cc, half * 512:(half + 1) * 512],
                        start=(cc == 0), stop=(cc == 3))
                nc.vector.tensor_copy(
                    osb_map[qc][:, half * 512:(half + 1) * 512], om[:])

            def emit_outdma(qc):
                nc.sync.dma_start(
                    out_d.ap()[qc * 128:(qc + 1) * 128, :], osb_map[qc][:])

            # ---------------- emission schedule ----------------
            # PE work other than the per-slot score matmuls is queued as
            # "filler" closures, drained one per score group so the PE
            # instruction stream never has a multi-us block that would
            # starve the ACT engine (scores feed exp, the bottleneck).
            import collections as _c
            pe_fill = _c.deque()

            def drain(n):
                while n > 0 and pe_fill:
                    pe_fill.popleft()()
                    n -= 1

            # prologue: QK projections + remaps for tokens 0:512, V chunks
            # 0-1 (everything level 0/1 needs); cc0 first so slot (0,0)
            # starts as early as possible.
            emit_qkproj(0, 0, "q")
            emit_qkproj(0, 0, "k")
            emit_remap(0, 0, 512)
            for cc in range(1, 4):
                pe_fill.append(lambda cc=cc: emit_qkproj(cc, 0, "q"))
                pe_fill.append(lambda cc=cc: emit_qkproj(cc, 0, "k"))
                pe_fill.append(lambda cc=cc: emit_remap(cc, 0, 512))
            pe_fill.append(lambda: emit_vproj(0))
            pe_fill.append(lambda: emit_vproj(1))
            drain(4)

            # remaining QK proj units: st1 fine-grained remap, st2+st3 with
            # a merged remap of tokens [1024:2048]; pushed one per 2 slots.
            qk_units = [("s1", cc) for cc in range(4)]
            qk_units += [("s23", cc) for cc in range(4)]
            v_units = list(range(2, 16))

            def push_qk(kind, cc):
                if kind == "s1":
                    pe_fill.append(lambda: emit_qkproj(cc, 1, "q"))
                    pe_fill.append(lambda: emit_qkproj(cc, 1, "k"))
                    pe_fill.append(lambda: emit_remap(cc, 512, 1024))
                else:
                    pe_fill.append(lambda: emit_qkproj(cc, 2, "q"))
                    pe_fill.append(lambda: emit_qkproj(cc, 2, "k"))
                    pe_fill.append(lambda: emit_qkproj(cc, 3, "q"))
                    pe_fill.append(lambda: emit_qkproj(cc, 3, "k"))
                    pe_fill.append(lambda: emit_remap(cc, 1024, 2048))

            for j in range(N_J):
                for h in range(8):
                    si = j * 8 + h
                    groups = group_chunks(j)
                    pts_map = {}
                    for gi, g in enumerate(groups):
                        st_t = emit_qk_scores(h, j, g)
                        pts_map[gi] = emit_exp(h, j, g, st_t)
                        drain(1)
                    emit_mask(h, j, pts_map)
                    # flush stragglers so pv/norm of slot s-1 lands in slot s
                    drain(len(pe_fill) - 3)
                    sl = (si % 2) * 256
                    pe_fill.append(
                        lambda h=h, j=j, p=pts_map, sl=sl:
                            emit_pv_qb(h, j, p, sl, 0))
                    pe_fill.append(
                        lambda h=h, j=j, p=pts_map, sl=sl:
                            emit_pv_qb(h, j, p, sl, 1))
                    pe_fill.append(
                        lambda h=h, j=j, sl=sl: emit_pv_fin(h, j, sl))
                    if j > 1 and h == 1:
                        # level j-2 fully normalized (flushed above); the
                        # extra level of lag keeps trans/out DMAs off the
                        # DMA-congested head region.
                        for qc in (2 * (j - 2), 2 * (j - 2) + 1):
                            emit_trans(qc)
                            for half in range(2):
                                pe_fill.append(
                                    lambda qc=qc, half=half:
                                        emit_om(qc, half))
                            pe_fill.append(lambda qc=qc: emit_outdma(qc))
                    if si % 2 == 1 and qk_units:
                        push_qk(*qk_units.pop(0))
                    if h in (2, 5) and v_units:
                        c = v_units.pop(0)
                        pe_fill.append(lambda c=c: emit_vproj(c))
            drain(len(pe_fill))
            for qc in (12, 13, 14, 15):
                emit_trans(qc)
                emit_om(qc, 0)
                emit_om(qc, 1)
                emit_outdma(qc)
    nc.compile()
    return nc


def _get_nc():
    if "nc" not in _CACHE:
        _CACHE["nc"] = _build_nc()
    return _CACHE["nc"]


def _split8(a, scale=1.0):
    hi = (a * scale).astype(ml_dtypes.float8_e4m3)
    lo = (a * scale - hi.astype(np.float32)).astype(ml_dtypes.float8_e5m2)
    return hi, lo


def _wlayout(Wt):
    # Wt: [1024 in, 512 out] -> [128 p, 4 c2, 2 i, 512 out] -> [128, 4096]
    a = Wt.reshape(4, 2, 128, 512).transpose(2, 0, 1, 3)
    return np.ascontiguousarray(a.reshape(128, 4096))


def make_in_maps(x, Wq, bq, Wk, bk, Wv, bv, Wo):
    """Host-side sharding: returns per-core input dicts (core = 2b + hh)."""
    xt = np.ascontiguousarray(
        np.transpose(np.asarray(x, np.float32), (0, 2, 1)))  # [B, H, S]
    xh, xl = _split8(xt)
    tri = np.triu(np.ones((128, 128), np.float32)).astype(ml_dtypes.bfloat16)

    packs = []
    for hh in range(2):
        r = slice(hh * C, (hh + 1) * C)
        wqh_, wql_ = _split8(_wlayout(np.asarray(Wq, np.float32)[r, :].T), WS)
        wkh_, wkl_ = _split8(_wlayout(np.asarray(Wk, np.float32)[r, :].T), WS)
        wvh_, wvl_ = _split8(_wlayout(np.asarray(Wv, np.float32)[r, :].T), WS)
        wo_ = np.asarray(Wo, np.float32)[:, r].T  # [512 in, 1024 out]
        wo_ = np.ascontiguousarray(
            wo_.reshape(4, 128, 1024).transpose(1, 0, 2).reshape(128, 4096)
        ).astype(ml_dtypes.bfloat16)
        bq_ = np.ascontiguousarray(
            np.asarray(bq, np.float32)[r].reshape(4, 128).T)
        bk_ = np.ascontiguousarray(
            np.asarray(bk, np.float32)[r].reshape(4, 128).T)
        packs.append({"wqh": wqh_, "wql": wql_, "wkh": wkh_, "wkl": wkl_,
                      "wvh": wvh_, "wvl": wvl_, "wo": wo_,
                      "bq": bq_, "bk": bk_})

    in_maps = []
    for c in range(NCORES):
        b, hh = c // 2, c % 2
        m = {"xh": xh[b], "xl": xl[b], "tri": tri}
        m.update(packs[hh])
        in_maps.append(m)
    return in_maps


def run_cores(in_maps):
    nc = _get_nc()
    res = run_bass_kernel_spmd(nc, in_maps, core_ids=list(range(NCORES)))
    return [r["out"] for r in res.results]


def host_combine(partials, bo, bv, Wo):
    out = np.zeros((B, S, H), np.float32)
    for b in range(B):
        out[b] = (np.asarray(partials[2 * b]).astype(np.float32)
                  + np.asarray(partials[2 * b + 1]).astype(np.float32))
    bias = (np.asarray(bo, np.float32)
            + np.asarray(bv, np.float32) @ np.asarray(Wo, np.float32).T)
    return out + bias[None, None, :]


def kernel(x, mask, Wq, bq, Wk, bk, Wv, bv, Wo, bo):
    in_maps = make_in_maps(x, Wq, bq, Wk, bk, Wv, bv, Wo)
    partials = run_cores(in_maps)
    return host_combine(partials, bo, bv, Wo).astype(np.float32)


# revision 19
# speedup vs baseline: 1.1665x; 1.0522x over previous
"""Multi-head causal attention (B=4, S=2048, H=1024, NH=16) on 8 trn2 cores.

Hybrid sharding: core = (batch b, head-half hh) -> 1 batch x 8 heads per
core.  fp8 DoubleRow matmuls carry the projections (hi/lo e4m3+e5m2
compensation: Q/K 2 passes, V 3) and the Q*K score matmuls (e4m3 stores);
P@V / output projection run bf16.  Scores are computed transposed
S^T[k,q] so softmax denominators fall out of a ones-column in the P@V
accumulation and normalization is a per-partition scalar multiply.  The
ctx^T -> ctx transpose uses the DMA xbar (no PE/PSUM involved), and the
output projection contracts all 512 local channels so each core emits a
[2048,1024] partial for its batch; the host sums the 2 partials per
batch and folds in bo + bv@Wo^T.  Slot order is j-outer/h-inner so
transposes + output projection pipeline level-by-level."""
import numpy as np
import ml_dtypes

import concourse.bacc as bacc
import concourse.tile as tile
from concourse import mybir
from concourse.bass_utils import run_bass_kernel_spmd

F32 = mybir.dt.float32
BF16 = mybir.dt.bfloat16
E4 = mybir.dt.float8e4
E5 = mybir.dt.float8e5
AF = mybir.ActivationFunctionType
DR = mybir.MatmulPerfMode.DoubleRow
MULT = mybir.AluOpType.mult
ADD = mybir.AluOpType.add

B, S, H, NH = 4, 2048, 1024, 16
HD = H // NH            # 64
NCORES = 8
HPC = 8                 # heads per core
C = HPC * HD            # 512 channels per core
SCALE = 1.0 / np.sqrt(HD)
WS = 32.0               # weight pre-scale (keeps fp8 hi in normal range)

N_J = S // 256          # 8 j-levels of 256 q each
QK_PASSES = 2
V_PASSES = 3

_CACHE = {}


def _build_nc():
    nc = bacc.Bacc(name="mha_v2")
    xh_d = nc.dram_tensor("xh", [H, S], E4, kind="ExternalInput")
    xl_d = nc.dram_tensor("xl", [H, S], E5, kind="ExternalInput")
    wqh_d = nc.dram_tensor("wqh", [128, 4096], E4, kind="ExternalInput")
    wql_d = nc.dram_tensor("wql", [128, 4096], E5, kind="ExternalInput")
    wkh_d = nc.dram_tensor("wkh", [128, 4096], E4, kind="ExternalInput")
    wkl_d = nc.dram_tensor("wkl", [128, 4096], E5, kind="ExternalInput")
    wvh_d = nc.dram_tensor("wvh", [128, 4096], E4, kind="ExternalInput")
    wvl_d = nc.dram_tensor("wvl", [128, 4096], E5, kind="ExternalInput")
    wo_d = nc.dram_tensor("wo", [128, 4096], BF16, kind="ExternalInput")
    bq_d = nc.dram_tensor("bq", [128, 4], F32, kind="ExternalInput")
    bk_d = nc.dram_tensor("bk", [128, 4], F32, kind="ExternalInput")
    tri_d = nc.dram_tensor("tri", [128, 128], BF16, kind="ExternalInput")
    out_d = nc.dram_tensor("out", [S, H], BF16, kind="ExternalOutput")

    with tile.TileContext(nc) as tc:
        with (
            tc.tile_pool(name="const", bufs=1) as cp,
            tc.tile_pool(name="xs", bufs=1) as xp,
            tc.tile_pool(name="qk", bufs=1) as qp,
            tc.tile_pool(name="vn", bufs=1) as vp,
            tc.tile_pool(name="pts", bufs=10) as pp,
            tc.tile_pool(name="ctx", bufs=1) as ctp,
            tc.tile_pool(name="small", bufs=8) as sp,
            tc.tile_pool(name="osb", bufs=3) as op_,
            tc.tile_pool(name="stp", bufs=2, space="PSUM") as stp,
            tc.tile_pool(name="accp", bufs=1, space="PSUM") as accp,
            tc.tile_pool(name="mixp", bufs=3, space="PSUM") as mixp,
        ):
            # ---- constants ----
            wqh_s = cp.tile([128, 4096], E4)
            wql_s = cp.tile([128, 4096], E5)
            wkh_s = cp.tile([128, 4096], E4)
            wkl_s = cp.tile([128, 4096], E5)
            wvh_s = cp.tile([128, 4096], E4)
            wvl_s = cp.tile([128, 4096], E5)
            wo_s = cp.tile([128, 4096], BF16)
            bq_s = cp.tile([128, 4], F32)
            bk_s = cp.tile([128, 4], F32)
            tri_s = cp.tile([128, 128], BF16)
            for s_, d_ in ((wqh_s, wqh_d), (wql_s, wql_d), (bq_s, bq_d),
                           (wkh_s, wkh_d), (wkl_s, wkl_d), (bk_s, bk_d),
                           (tri_s, tri_d)):
                nc.sync.dma_start(s_[:], d_.ap())

            # x tiles (one batch): [128 p, 8 c, 2048 t], ch = c*128 + p
            xh_s = xp.tile([128, 16384], E4)
            xl_s = xp.tile([128, 16384], E5)

            def emit_xload(s_, d_, q):
                qsl = slice(q * 512, (q + 1) * 512)
                nc.sync.dma_start(
                    s_.rearrange("p (c t) -> p c t", c=8)[:, :, qsl],
                    d_.ap()[:, qsl].rearrange("(c p) t -> p c t", p=128))

            emit_xload(xh_s, xh_d, 0)
            emit_xload(xl_s, xl_d, 0)
            for s_, d_ in ((wvh_s, wvh_d), (wvl_s, wvl_d)):
                nc.sync.dma_start(s_[:], d_.ap())
            for q in range(1, 4):
                emit_xload(xh_s, xh_d, q)
            for q in range(1, 4):
                emit_xload(xl_s, xl_d, q)
            nc.sync.dma_start(wo_s[:], wo_d.ap())

            # weight views [p, c2, i, out512]
            wv_ = {
                "qh": wqh_s.rearrange("p (c i o) -> p c i o", c=4, i=2),
                "ql": wql_s.rearrange("p (c i o) -> p c i o", c=4, i=2),
                "kh": wkh_s.rearrange("p (c i o) -> p c i o", c=4, i=2),
                "kl": wkl_s.rearrange("p (c i o) -> p c i o", c=4, i=2),
                "vh": wvh_s.rearrange("p (c i o) -> p c i o", c=4, i=2),
                "vl": wvl_s.rearrange("p (c i o) -> p c i o", c=4, i=2),
            }
            wo_v = wo_s.rearrange("p (cc o) -> p cc o", cc=4)
            xh_v = xh_s.rearrange("p (c i t) -> p c i t", c=4, i=2)
            xl_v = xl_s.rearrange("p (c i t) -> p c i t", c=4, i=2)

            # q/k channel-major tiles per cc chunk (2 heads each)
            qt = [qp.tile([128, 2048], E4, name=f"qt{cc}") for cc in range(4)]
            kt = [qp.tile([128, 2048], E4, name=f"kt{cc}") for cc in range(4)]
            # DR-packed q/k: [64 (2h x 32), 2 i, 2048 t] per head-pair cc
            qdr = [qp.tile([64, 4096], E4, name=f"qdr{g}") for g in range(4)]
            kdr = [qp.tile([64, 4096], E4, name=f"kdr{g}") for g in range(4)]
            # v (+ones col): [128 k-part, 8 h, 16 kc, 65]
            vn = vp.tile([128, 8320], BF16)
            vn4 = vn.rearrange("p (h c e) -> p h c e", h=8, e=65)
            nc.vector.memset(vn4[:, :, :, 64], 1.0)
            # ctx^T [128 tok, 16 qc, 8 h, 64 d]; ctx [128 chp, 4 cc, 2048 t]
            ctxT = ctp.tile([128, 8192], BF16)
            ctx = ctp.tile([128, 8192], BF16)
            ctx3 = ctx.rearrange("p (cc t) -> p cc t", cc=4)
            acc = accp.tile([128, 512], F32)
            accv = acc.rearrange("p (s q e) -> p s q e", s=2, q=2)

            def emit_qkproj(cc, st, which):
                tsl = slice(st * 512, (st + 1) * 512)
                wh, wl, bias, dst = (("qh", "ql", bq_s, qt) if which == "q"
                                     else ("kh", "kl", bk_s, kt))
                pmm = mixp.tile([128, 512], F32, tag="mix",
                                name=f"pp{wh}{cc}_{st}")
                first = True
                for w_v in (wv_[wh], wv_[wl])[:QK_PASSES]:
                    for c2 in range(4):
                        nc.tensor.matmul(
                            pmm[:],
                            w_v[:, c2, :, cc * 128:(cc + 1) * 128],
                            xh_v[:, c2, :, tsl],
                            start=first,
                            stop=(w_v is wv_[wl] and c2 == 3),
                            perf_mode=DR)
                        first = False
                nc.vector.tensor_scalar(dst[cc][:, tsl], pmm[:], 1.0 / WS,
                                        bias[:, cc:cc + 1], op0=MULT,
                                        op1=ADD)

            remap_rr = [0]

            def emit_remap(cc, t0, t1):
                # round-robin the remap DMAs over the SP/ACT HWDGE queues
                # and the Pool SWDGE queue: pushing all of them through one
                # serialized DGE device walls the first ~100us.
                tsl = slice(t0, t1)
                for src_l, dr in ((qt, qdr), (kt, kdr)):
                    drv = dr[cc].rearrange("p (i t) -> p i t", i=2)
                    for hl in range(2):
                        p0 = hl * 32
                        for i in range(2):
                            eng = (nc.sync, nc.gpsimd,
                                   nc.scalar)[remap_rr[0] % 3]
                            remap_rr[0] += 1
                            eng.dma_start(
                                drv[p0:p0 + 32, i, tsl],
                                src_l[cc][hl * 64 + i * 32:
                                          hl * 64 + i * 32 + 32, tsl])

            def emit_vproj(c):
                # flipped V: psum [128 tok, 512 ch] for one 128-token chunk
                tsl = slice(c * 128, (c + 1) * 128)
                vm = mixp.tile([128, 512], F32, tag="mix", name=f"vp{c}")
                passes = [("vh", xh_v), ("vl", xh_v)]
                if V_PASSES >= 3:
                    passes.append(("vh", xl_v))
                first = True
                for wn, x_v in passes:
                    for c2 in range(4):
                        nc.tensor.matmul(
                            vm[:], x_v[:, c2, :, tsl], wv_[wn][:, c2],
                            start=first,
                            stop=(wn == passes[-1][0] and x_v is passes[-1][1]
                                  and c2 == 3),
                            perf_mode=DR)
                        first = False
                nc.vector.tensor_scalar(
                    vn4[:, :, c, 0:64],
                    vm.rearrange("p (h d) -> p h d", h=8), 1.0 / WS, None,
                    op0=MULT)

            def group_chunks(j):
                nkc = 2 * (j + 1)
                groups = []
                for g0 in range(0, nkc, 4):
                    cs = list(range(g0, min(g0 + 4, nkc)))
                    pos, w = [], []
                    for c in cs:
                        pos.append((c - g0) * 256)
                        w.append(128 if c == nkc - 1 else 256)
                    groups.append((cs, w, pos, pos[-1] + w[-1]))
                return groups

            def emit_qk_scores(h, j, g):
                qv = qdr[h // 2].rearrange("p (i t) -> p i t", i=2)
                kv = kdr[h // 2].rearrange("p (i t) -> p i t", i=2)
                hsl = slice((h % 2) * 32, (h % 2) * 32 + 32)
                cs, ws, poss, tw = g
                st = stp.tile([128, 1024], F32, tag="st",
                              name=f"st{h}_{j}_{cs[0]}")
                nkc = 2 * (j + 1)
                for c, w, pos in zip(cs, ws, poss):
                    q0 = j * 256 + (128 if c == nkc - 1 else 0)
                    nc.tensor.matmul(
                        st[:, pos:pos + w],
                        kv[hsl, :, c * 128:(c + 1) * 128],
                        qv[hsl, :, q0:q0 + w],
                        start=True, stop=True, perf_mode=DR)
                return st

            def emit_exp(h, j, g, st):
                cs, ws, poss, tw = g
                pt = pp.tile([128, 1024], BF16, tag="pt",
                             name=f"pt{h}_{j}_{cs[0]}")
                nc.scalar.activation(pt[:, 0:tw], st[:, 0:tw], AF.Exp,
                                     scale=float(SCALE))
                return pt

            def emit_mask(h, j, pts_map):
                nkc = 2 * (j + 1)
                for c in (nkc - 2, nkc - 1):
                    pt = pts_map[c // 4]
                    pos = (c % 4) * 256
                    nc.gpsimd.tensor_mul(pt[:, pos:pos + 128],
                                         pt[:, pos:pos + 128], tri_s[:])

            def emit_pv_qb(h, j, pts_map, sl, qb):
                nkc = 2 * (j + 1)
                qc = 2 * j + qb
                for c in range(qc + 1):
                    pt = pts_map[c // 4]
                    pos = (c % 4) * 256 + (qb * 128 if c < nkc - 1 else 0)
                    nc.tensor.matmul(
                        acc[:, sl + qb * 128: sl + qb * 128 + 65],
                        pt[:, pos:pos + 128],
                        vn[:, h * 1040 + c * 65: h * 1040 + (c + 1) * 65],
                        start=(c == 0), stop=(c == qc),
                        skip_group_check=True)

            def emit_pv_fin(h, j, sl):
                den = sp.tile([128, 2], F32, tag="den", name=f"dn{h}_{j}")
                s2 = sl // 256
                nc.vector.reciprocal(den[:], accv[:, s2, :, 64])
                for qb in range(2):
                    qc = 2 * j + qb
                    nc.vector.tensor_scalar(
                        ctxT[:, qc * 512 + h * 64: qc * 512 + (h + 1) * 64],
                        acc[:, sl + qb * 128: sl + qb * 128 + 64],
                        den[:, qb:qb + 1], None, op0=MULT)

            def emit_trans(qc):
                nc.sync.dma_start_transpose(
                    ctx3[:, :, qc * 128:(qc + 1) * 128],
                    ctxT[:, qc * 512:(qc + 1) * 512])

            osb_map = {}

            def emit_om(qc, half):
                if qc not in osb_map:
                    osb_map[qc] = op_.tile([128, 1024], BF16, tag="osb",
                                           name=f"ob{qc}")
                om = mixp.tile([128, 512], F32, tag="mix",
                               name=f"om{qc}_{half}")
                for cc in range(4):
                    nc.tensor.matmul(
                        om[:],
                        ctx3[:, cc, qc * 128:(qc + 1) * 128],
                        wo_v[:, cc, half * 512:(half + 1) * 512],
                        start=(cc == 0), stop=(cc == 3))
                nc.vector.tensor_copy(
                    osb_map[qc][:, half * 512:(half + 1) * 512], om[:])

            def emit_outdma(qc):
                nc.sync.dma_start(
                    out_d.ap()[qc * 128:(qc + 1) * 128, :], osb_map[qc][:])

            # ---------------- emission schedule ----------------
            # PE work other than the per-slot score matmuls is queued as
            # "filler" closures, drained one per score group so the PE
            # instruction stream never has a multi-us block that would
            # starve the ACT engine (scores feed exp, the bottleneck).
            # Each entry is (level_tag, closure); level_tag marks the pv/
            # norm chain of that attention level so emit_trans can force
            # the prerequisite normalizations out of the queue.
            import collections as _c
            pe_fill = _c.deque()

            def drain(n):
                while n > 0 and pe_fill:
                    pe_fill.popleft()[1]()
                    n -= 1

            def drain_level(lv):
                while any(t is not None and t <= lv for t, _ in pe_fill):
                    pe_fill.popleft()[1]()

            # prologue: cc0 Q/K over tokens 0:1024 (covers levels 0-3) +
            # remap; cc1-3, V0/V1 queued as fillers.
            for st in (0, 1):
                emit_qkproj(0, st, "q")
                emit_qkproj(0, st, "k")
            emit_remap(0, 0, 1024)
            for cc in range(1, 4):
                for st in (0, 1):
                    pe_fill.append(
                        (None, lambda cc=cc, st=st: emit_qkproj(cc, st, "q")))
                    pe_fill.append(
                        (None, lambda cc=cc, st=st: emit_qkproj(cc, st, "k")))
                pe_fill.append((None, lambda cc=cc: emit_remap(cc, 0, 1024)))
                if cc == 1:
                    pe_fill.append((None, lambda: emit_vproj(0)))
                    pe_fill.append((None, lambda: emit_vproj(1)))
            drain(3)

            # remaining QK proj (tokens 1024:2048, needed from level 4 /
            # slot 32): one unit per 2 slots starting at slot 1.
            qk_units = list(range(4))
            v_units = list(range(2, 16))

            def push_qk(cc):
                for st in (2, 3):
                    pe_fill.append(
                        (None, lambda cc=cc, st=st: emit_qkproj(cc, st, "q")))
                    pe_fill.append(
                        (None, lambda cc=cc, st=st: emit_qkproj(cc, st, "k")))
                pe_fill.append(
                    (None, lambda cc=cc: emit_remap(cc, 1024, 2048)))

            for j in range(N_J):
                for h in range(8):
                    si = j * 8 + h
                    groups = group_chunks(j)
                    pts_map = {}
                    for gi, g in enumerate(groups):
                        st_t = emit_qk_scores(h, j, g)
                        pts_map[gi] = emit_exp(h, j, g, st_t)
                        drain(1)
                    emit_mask(h, j, pts_map)
                    drain(2)
                    sl = (si % 2) * 256
                    pe_fill.append(
                        (j, lambda h=h, j=j, p=pts_map, sl=sl:
                            emit_pv_qb(h, j, p, sl, 0)))
                    pe_fill.append(
                        (j, lambda h=h, j=j, p=pts_map, sl=sl:
                            emit_pv_qb(h, j, p, sl, 1)))
                    pe_fill.append(
                        (j, lambda h=h, j=j, sl=sl: emit_pv_fin(h, j, sl)))
                    if j > 1 and h == 1:
                        # level j-2: force its pv/norm chain out, then
                        # transpose + output-project it.
                        drain_level(j - 2)
                        for qc in (2 * (j - 2), 2 * (j - 2) + 1):
                            emit_trans(qc)
                            for half in range(2):
                                pe_fill.append(
                                    (None, lambda qc=qc, half=half:
                                        emit_om(qc, half)))
                            pe_fill.append(
                                (None, lambda qc=qc: emit_outdma(qc)))
                    if si % 2 == 1 and qk_units:
                        push_qk(qk_units.pop(0))
                    if h in (2, 5) and v_units:
                        c = v_units.pop(0)
                        pe_fill.append(
                            (None, lambda c=c: emit_vproj(c)))
            drain(len(pe_fill))
            for qc in (12, 13, 14, 15):
                emit_trans(qc)
                emit_om(qc, 0)
                emit_om(qc, 1)
                emit_outdma(qc)
    nc.compile()
    return nc


def _get_nc():
    if "nc" not in _CACHE:
        _CACHE["nc"] = _build_nc()
    return _CACHE["nc"]


def _split8(a, scale=1.0):
    hi = (a * scale).astype(ml_dtypes.float8_e4m3)
    lo = (a * scale - hi.astype(np.float32)).astype(ml_dtypes.float8_e5m2)
    return hi, lo


def _wlayout(Wt):
    # Wt: [1024 in, 512 out] -> [128 p, 4 c2, 2 i, 512 out] -> [128, 4096]
    a = Wt.reshape(4, 2, 128, 512).transpose(2, 0, 1, 3)
    return np.ascontiguousarray(a.reshape(128, 4096))


def make_in_maps(x, Wq, bq, Wk, bk, Wv, bv, Wo):
    """Host-side sharding: returns per-core input dicts (core = 2b + hh)."""
    xt = np.ascontiguousarray(
        np.transpose(np.asarray(x, np.float32), (0, 2, 1)))  # [B, H, S]
    xh, xl = _split8(xt)
    tri = np.triu(np.ones((128, 128), np.float32)).astype(ml_dtypes.bfloat16)

    packs = []
    for hh in range(2):
        r = slice(hh * C, (hh + 1) * C)
        wqh_, wql_ = _split8(_wlayout(np.asarray(Wq, np.float32)[r, :].T), WS)
        wkh_, wkl_ = _split8(_wlayout(np.asarray(Wk, np.float32)[r, :].T), WS)
        wvh_, wvl_ = _split8(_wlayout(np.asarray(Wv, np.float32)[r, :].T), WS)
        wo_ = np.asarray(Wo, np.float32)[:, r].T  # [512 in, 1024 out]
        wo_ = np.ascontiguousarray(
            wo_.reshape(4, 128, 1024).transpose(1, 0, 2).reshape(128, 4096)
        ).astype(ml_dtypes.bfloat16)
        bq_ = np.ascontiguousarray(
            np.asarray(bq, np.float32)[r].reshape(4, 128).T)
        bk_ = np.ascontiguousarray(
            np.asarray(bk, np.float32)[r].reshape(4, 128).T)
        packs.append({"wqh": wqh_, "wql": wql_, "wkh": wkh_, "wkl": wkl_,
                      "wvh": wvh_, "wvl": wvl_, "wo": wo_,
                      "bq": bq_, "bk": bk_})

    in_maps = []
    for c in range(NCORES):
        b, hh = c // 2, c % 2
        m = {"xh": xh[b], "xl": xl[b], "tri": tri}
        m.update(packs[hh])
        in_maps.append(m)
    return in_maps


def run_cores(in_maps):
    nc = _get_nc()
    res = run_bass_kernel_spmd(nc, in_maps, core_ids=list(range(NCORES)))
    return [r["out"] for r in res.results]


def host_combine(partials, bo, bv, Wo):
    out = np.zeros((B, S, H), np.float32)
    for b in range(B):
        out[b] = (np.asarray(partials[2 * b]).astype(np.float32)
                  + np.asarray(partials[2 * b + 1]).astype(np.float32))
    bias = (np.asarray(bo, np.float32)
            + np.asarray(bv, np.float32) @ np.asarray(Wo, np.float32).T)
    return out + bias[None, None, :]


def kernel(x, mask, Wq, bq, Wk, bk, Wv, bv, Wo, bo):
    in_maps = make_in_maps(x, Wq, bq, Wk, bk, Wv, bv, Wo)
    partials = run_cores(in_maps)
    return host_combine(partials, bo, bv, Wo).astype(np.float32)
